# revision 1
# baseline (speedup 1.0000x reference)
"""Chamfer loss (brute-force, no sigma) on 8 trn2 NeuronCores.

Strategy (data-parallel over batch, one batch element per core):
  negsq[m,n] = -|src_m - dst_n|^2 is produced by an augmented matmul
  (K=18 rows of exact bf16 splits) so PSUM holds the NEGATED squared
  distance; every reduction is then a MAX.  Per 128-row block:
    - ScalarE evacuates the two 2048-col PSUM chunks to bf16 SBUF (the
      per-block pace-setter: 2 x (2048+222) cycles at 1.2 GHz).
    - fwd (src->dst): one fused 4x-mode tensor_scalar max-reduce on
      VectorE -> facc[:, i].
    - bwd (dst->src): running elementwise max into bacc[128, N] via one
      2x-mode tensor_tensor on VectorE.  (GPSIMD tensor_tensor fails
      walrus codegen in this toolchain, so VectorE carries it all; its
      fwd+bwd total still fits under the ScalarE evacuation time.)
  Software-pipelined emission: block i's evacuation ops are enqueued
  before block i-1's fwd/bwd so PSUM tiles free up early.  The final
  block streams bacc quarters to DRAM as they finish; the host does the
  128-way cross-partition max, the sqrt and the means.  Startup: GPSIMD
  memsets bacc while dummy matmuls warm the PE p-state ramp.
"""

import numpy as np
import ml_dtypes
from contextlib import ExitStack

B, C = 8, 3
M = N = 4096
NCORES = 8
PB = 128          # output partition block (m rows per matmul)
KAUG = 18         # augmented contraction dim
BIG = 3.0e4       # > max possible squared distance
MMN = 512         # matmul moving free dim (one fp32 PSUM bank)
PW = 2048         # psum chunk width (fp32, 4 banks)
VC = 0            # per-chunk columns evacuated by VectorE (rest: ScalarE)
PRIO = 20         # priority boost for the PSUM-reading DVE copies
SB_BUFS = 3
VERSION = "maxlane-v16"

bf16np = ml_dtypes.bfloat16


# ----------------------------------------------------------------------------
# Device program
# ----------------------------------------------------------------------------

def _body(ctx, tc, lhs, rhs, rowneg_d, colneg_d, m, n, vc=VC, reps=1):
    import concourse.mybir as mybir

    nc = tc.nc
    f32 = mybir.dt.float32
    bf16 = mybir.dt.bfloat16
    MAX = mybir.AluOpType.max

    nblk = m // PB
    nch = n // PW

    cpool = ctx.enter_context(tc.tile_pool(name="const", bufs=1))
    ppool = ctx.enter_context(tc.tile_pool(name="psum", bufs=2, space="PSUM"))
    spool = ctx.enter_context(tc.tile_pool(name="sb", bufs=SB_BUFS))
    jpool = ctx.enter_context(tc.tile_pool(name="junk", bufs=2))

    # Load block-0 stationary columns first so matmuls can start ASAP.
    lhs_t = cpool.tile([KAUG, m], bf16)
    nc.sync.dma_start(out=lhs_t[:, :PB], in_=lhs[:, :PB])
    rhs_t = cpool.tile([KAUG, n], bf16)
    nc.sync.dma_start(out=rhs_t[:, :PW], in_=rhs[:, :PW])
    nc.sync.dma_start(out=rhs_t[:, PW:], in_=rhs[:, PW:])
    nc.sync.dma_start(out=lhs_t[:, PB:], in_=lhs[:, PB:])

    facc = cpool.tile([PB, nblk], f32)
    bacc = cpool.tile([PB, n], bf16)
    nc.gpsimd.memset(bacc[:], -BIG)

    # Warm the PE p-state ramp while input DMAs land: tiny matmuls against a
    # memset tile so they have no DMA dependency.
    wconst = cpool.tile([KAUG, PB], bf16)
    nc.vector.memset(wconst[:], 0.0)
    wt = ppool.tile([PB, PW], f32, tag="pt")
    for _ in range(40):
        nc.tensor.matmul(wt[:, :1], wconst[:], wconst[:, :1],
                         start=True, stop=True)

    NQ = 4                    # final-block bwd/output quarters

    def emit_compute(i, sb, last=False):
        if not last:
            # fwd row-max in one fused 4x tensor_scalar reduce
            junk = jpool.tile([PB, n], bf16, tag="junk")
            nc.vector.tensor_scalar(junk[:], sb[:], float(-BIG), None,
                                    MAX, MAX, accum_out=facc[:, i:i + 1])
            nc.vector.tensor_tensor(bacc[:], bacc[:], sb[:], MAX)
        else:
            # Final block: bwd quarters first (each gated only on its own
            # evacuation chunk) streaming finished bacc slices to DRAM; the
            # host does the cross-partition max. fwd last, overlapping DMAs.
            q = n // NQ
            for k in range(NQ):
                sl = slice(k * q, (k + 1) * q)
                nc.vector.tensor_tensor(bacc[:, sl], bacc[:, sl],
                                        sb[:, sl], MAX)
                nc.sync.dma_start(out=colneg_d[:, sl], in_=bacc[:, sl])
            junk = jpool.tile([PB, n], bf16, tag="junk")
            nc.vector.tensor_scalar(junk[:], sb[:], float(-BIG), None,
                                    MAX, MAX, accum_out=facc[:, i:i + 1])

    for rep in range(reps):
        prev = None
        for i in range(nblk):
            sb = spool.tile([PB, n], bf16, tag="sb")
            for ch in range(nch):
                pt = ppool.tile([PB, PW], f32, tag="pt")
                for q in range(PW // MMN):
                    n0 = ch * PW + q * MMN
                    nc.tensor.matmul(
                        pt[:, q * MMN:(q + 1) * MMN],
                        lhs_t[:, i * PB:(i + 1) * PB],
                        rhs_t[:, n0:n0 + MMN],
                        start=True, stop=True,
                    )
                # evacuation split: ScalarE [0:PW-vc), VectorE [PW-vc:PW)
                nc.scalar.copy(sb[:, ch * PW:(ch + 1) * PW - vc],
                               pt[:, :PW - vc])
                if ch == nch - 1:
                    # compute for the PREVIOUS block goes between this
                    # block's ch0 and ch1 DVE copies: the in-order DVE
                    # sequencer then never head-of-line blocks on a copy
                    # whose matmuls haven't finished yet, while PSUM tiles
                    # still free up in time for the next block.
                    if prev is not None:
                        emit_compute(*prev)
                if vc > 0:
                    # High priority: the PSUM-reading copy must sort ahead
                    # of the previous block's fwd/bwd in the DVE queue so
                    # the PSUM tile frees before the next matmuls need it.
                    with tc.high_priority(offset=PRIO):
                        nc.vector.tensor_copy(sb[:, (ch + 1) * PW - vc:
                                                 (ch + 1) * PW],
                                              pt[:, PW - vc:])
            prev = (i, sb)
        emit_compute(*prev, last=(rep == reps - 1))

    nc.sync.dma_start(out=rowneg_d[:], in_=facc[:])


def build_nc(m=M, n=N, vc=VC, reps=1):
    import concourse.tile as tile
    import concourse.bacc as bacc_mod
    import concourse.mybir as mybir

    f32 = mybir.dt.float32
    bf16 = mybir.dt.bfloat16
    nblk = m // PB
    assert 0 <= vc < PW

    nc = bacc_mod.Bacc("TRN2", target_bir_lowering=False, debug=False)
    lhs = nc.dram_tensor("lhs_aug", [KAUG, m], bf16, kind="ExternalInput").ap()
    rhs = nc.dram_tensor("rhs_aug", [KAUG, n], bf16, kind="ExternalInput").ap()
    rowneg_d = nc.dram_tensor("rowneg", [PB, nblk], f32,
                              kind="ExternalOutput").ap()
    colneg_d = nc.dram_tensor("colneg", [PB, n], bf16,
                              kind="ExternalOutput").ap()
    with tile.TileContext(nc) as tc:
        with ExitStack() as ctx:
            _body(ctx, tc, lhs, rhs, rowneg_d, colneg_d, m, n,
                  vc=vc, reps=reps)
    nc.compile()
    return nc


# ----------------------------------------------------------------------------
# Host-side input prep: exact bf16 splits for the augmented operands.
# The augmented product is the NEGATED squared distance:
#   -sq = 2*s.d - |s|^2 - |d|^2
# ----------------------------------------------------------------------------

def _split2(x):
    hi = x.astype(bf16np).astype(np.float64)
    lo = (x - hi).astype(bf16np).astype(np.float64)
    return hi, lo


def _split3(x):
    h = x.astype(bf16np).astype(np.float64)
    r = x - h
    mdl = r.astype(bf16np).astype(np.float64)
    l = (r - mdl).astype(bf16np).astype(np.float64)
    return h, mdl, l


def prep_inputs(pc_src, pc_dst):
    """Build per-batch augmented operands L, R: [B, 18, M/N] bf16."""
    s = np.asarray(pc_src, dtype=np.float64)   # [B, 3, M]
    d = np.asarray(pc_dst, dtype=np.float64)   # [B, 3, N]
    b = s.shape[0]
    m = s.shape[2]
    n = d.shape[2]

    s_hi, s_lo = _split2(s)
    d_hi, d_lo = _split2(d)
    s2 = ((s_hi + s_lo) ** 2).sum(axis=1)      # [B, M]
    d2 = ((d_hi + d_lo) ** 2).sum(axis=1)      # [B, N]
    s2h, s2m, s2l = _split3(-s2)
    d2h, d2m, d2l = _split3(-d2)

    L = np.zeros((b, KAUG, m), dtype=np.float64)
    R = np.zeros((b, KAUG, n), dtype=np.float64)
    L[:, 0:3] = 2.0 * s_hi
    R[:, 0:3] = d_hi
    L[:, 3:6] = 2.0 * s_hi
    R[:, 3:6] = d_lo
    L[:, 6:9] = 2.0 * s_lo
    R[:, 6:9] = d_hi
    L[:, 9:12] = 2.0 * s_lo
    R[:, 9:12] = d_lo
    L[:, 12:15] = 1.0
    R[:, 12] = d2h
    R[:, 13] = d2m
    R[:, 14] = d2l
    L[:, 15] = s2h
    L[:, 16] = s2m
    L[:, 17] = s2l
    R[:, 15:18] = 1.0
    return L.astype(bf16np), R.astype(bf16np)


# ----------------------------------------------------------------------------
# Cached PJRT runner (compile once, execute many)
# ----------------------------------------------------------------------------

_STATE = {}


def _get_runner(reps=1):
    key = (reps, VERSION, VC, SB_BUFS)
    if key in _STATE:
        return _STATE[key]

    import jax
    from jax.experimental.shard_map import shard_map
    from jax.sharding import Mesh, PartitionSpec
    from concourse import bass2jax, mybir

    nc = build_nc(M, N, reps=reps)
    bass2jax.install_neuronx_cc_hook()

    in_names, out_names, out_avals = [], [], []
    for alloc in nc.m.functions[0].allocations:
        if not isinstance(alloc, mybir.MemoryLocationSet):
            continue
        name = alloc.memorylocations[0].name
        if alloc.kind == "ExternalInput":
            in_names.append(name)
        elif alloc.kind == "ExternalOutput":
            out_names.append(name)
            out_avals.append(jax.core.ShapedArray(
                tuple(alloc.tensor_shape), mybir.dt.np(alloc.dtype)))
    n_params = len(in_names)
    n_outs = len(out_names)
    all_in_names = tuple(in_names + out_names)
    donate = tuple(range(n_params, n_params + n_outs))

    def _jbody(*args):
        outs = bass2jax._bass_exec_p.bind(
            *args,
            out_avals=tuple(out_avals),
            in_names=all_in_names,
            out_names=tuple(out_names),
            lowering_input_output_aliases=(),
            sim_require_finite=True,
            sim_require_nnan=True,
            nc=nc,
        )
        return tuple(outs)

    devices = jax.devices()[:NCORES]
    mesh = Mesh(np.asarray(devices), ("core",))
    in_specs = (PartitionSpec("core"),) * (n_params + n_outs)
    out_specs = (PartitionSpec("core"),) * n_outs
    fn = jax.jit(
        shard_map(_jbody, mesh=mesh, in_specs=in_specs, out_specs=out_specs,
                  check_rep=False),
        donate_argnums=donate, keep_unused=True,
    )
    st = dict(fn=fn, nc=nc, in_names=in_names, out_names=out_names,
              out_avals=out_avals, n_params=n_params)
    _STATE[key] = st
    return st


def run_device(L, R, reps=1, _retry=True):
    """L, R: [NCORES, 18, M] bf16. Returns (rowneg [NCORES,128,32] fp32,
    colneg [NCORES,128,N] bf16) holding negsq maxima; colneg still needs
    the host-side cross-partition max."""
    st = _get_runner(reps)
    concat_in = []
    for name in st["in_names"]:
        arr = L if name == "lhs_aug" else R
        concat_in.append(np.concatenate([arr[c] for c in range(NCORES)], axis=0))
    concat_zeros = [
        np.zeros((NCORES * av.shape[0], *av.shape[1:]), av.dtype)
        for av in st["out_avals"]
    ]
    try:
        out_arrs = st["fn"](*concat_in, *concat_zeros)
        out_np = [np.asarray(a) for a in out_arrs]
    except Exception:
        # The shared axon terminal occasionally reports a transient
        # device-unrecoverable state; it clears after a short pause.
        if not _retry:
            raise
        import time as _time
        _time.sleep(20.0)
        return run_device(L, R, reps=reps, _retry=False)
    outs = {}
    for i, name in enumerate(st["out_names"]):
        av = st["out_avals"][i]
        outs[name] = out_np[i].reshape(NCORES, *av.shape)
    return outs["rowneg"], outs["colneg"]


# ----------------------------------------------------------------------------
# Public entry point
# ----------------------------------------------------------------------------

def _host_reduce(rowneg, colneg):
    # rowneg: [B, 128, M/128] fp32; colneg: [B, 128, N] bf16 (needs the
    # cross-partition max). Both hold -sq so min-sq = -max.
    rowsq = np.maximum(-rowneg.astype(np.float64), 0.0)
    colsq = np.maximum(-colneg.astype(np.float64).max(axis=1), 0.0)
    fwd = np.sqrt(rowsq).mean()
    bwd = np.sqrt(colsq).mean()
    total = np.float32(fwd + bwd)
    return total


def kernel(pc_src, pc_dst):
    L, R = prep_inputs(pc_src, pc_dst)
    rowneg, colneg = run_device(L, R)
    total = _host_reduce(rowneg, colneg)
    return (total, total, total)

